# revision 1
# baseline (speedup 1.0000x reference)
"""Binarized linear kernel for Trainium2 (8 NeuronCores, SPMD).

Computes out = x @ sign(weight).T with
  x:      [8192, 4096] f32
  weight: [4096, 4096] f32
  out:    [8192, 4096] f32

Strategy (data-parallel over M, per the sharding hint's second option):
  - Host: cast x/weight to bf16 (sign() is exact under the cast; x loses
    <2^-9 relative, well inside the matmul tolerance), pre-transpose both
    so the contraction dim K lands on SBUF partitions without any
    on-device transposes (fp32/bf16 transposed loads are slow paths).
  - Each core c gets xT shard [K, 1024] (columns c*1024:(c+1)*1024 of
    xT) resident in SBUF, and streams the full wT [K, 4096] once,
    binarizing tiles on the Scalar engine (Sign activation) on the fly.
  - PE: for each (n_tile of 512, m_tile of 128): accumulate 32 matmuls
    (K=4096 in chunks of 128) into one PSUM bank, evict via DVE copy,
    DMA the [128, 512] f32 block to the output shard.
  - Gather: concatenate the 8 per-core [1024, 4096] outputs over M.
"""

import os
import sys

import numpy as np

# Toolchain locations (normally already on sys.path via PYTHONPATH; be
# robust when invoked from a fresh directory/environment).
for _p in (
    "/root/.axon_site",
    "/root/.axon_site/_ro/trn_rl_repo",
    "/root/.axon_site/_ro/pypackages",
    "/opt/trn_rl_repo",
):
    if os.path.isdir(_p) and _p not in sys.path:
        sys.path.append(_p)

import ml_dtypes  # noqa: E402

BF16 = ml_dtypes.bfloat16

M, K, N = 8192, 4096, 4096
N_CORES = 8
P = 128
N_TILE = 512


def build_nc(mc: int = M // N_CORES, k: int = K, n: int = N):
    """Build the per-core Bass program. Same program runs SPMD on all
    cores; only the input data differs."""
    from concourse import bacc, mybir, tile

    ko_cnt = k // P
    mj_cnt = mc // P
    nt_cnt = n // N_TILE

    nc = bacc.Bacc("TRN2", target_bir_lowering=False)

    xT = nc.dram_tensor("xT", [k, mc], mybir.dt.bfloat16, kind="ExternalInput")
    wT = nc.dram_tensor("wT", [k, n], mybir.dt.bfloat16, kind="ExternalInput")
    out = nc.dram_tensor("out", [mc, n], mybir.dt.float32, kind="ExternalOutput")

    xT_ap = xT[:].rearrange("(ko p) m -> p ko m", p=P)
    wT_ap = wT[:].rearrange("(ko p) n -> p ko n", p=P)
    out_ap = out[:].rearrange("(t p) n -> t p n", p=P)

    n_warm = 20 if mc >= 1024 else 0

    with tile.TileContext(nc) as tc:
        with (
            tc.tile_pool(name="xres", bufs=1) as xpool,
            tc.tile_pool(name="warmp", bufs=1) as warmpool,
            tc.tile_pool(name="w", bufs=3) as wpool,
            tc.tile_pool(name="o", bufs=4) as opool,
            tc.tile_pool(name="ps", bufs=8, space="PSUM") as pspool,
        ):
            # HAM warm-up: dummy matmuls on a zeroed tile fill the ~12us
            # of dead PE time while the prologue + first DMAs run, so the
            # real matmul stream starts at 2.4 GHz instead of 1.2.
            if n_warm:
                warm = warmpool.tile([P, N_TILE], mybir.dt.bfloat16)
                nc.gpsimd.memset(warm[:], 0)
                warm_ps = pspool.tile([P, N_TILE], mybir.dt.float32, tag="ps")
                for _ in range(n_warm):
                    nc.tensor.matmul(
                        warm_ps[:], warm[:, :P], warm[:], start=True, stop=True
                    )

            x_res = xpool.tile([P, ko_cnt, mc], mybir.dt.bfloat16)

            def load_w(nt, interleave_x=False):
                w_tile = wpool.tile([P, ko_cnt, N_TILE], mybir.dt.bfloat16)
                nsl = slice(nt * N_TILE, (nt + 1) * N_TILE)
                for ko in range(ko_cnt):
                    if interleave_x:
                        if ko < 2:
                            # Halve the first chunks: lower arrival latency
                            # for the very first matmuls during the ramp.
                            h = mc // 2
                            nc.sync.dma_start(x_res[:, ko, :h], xT_ap[:, ko, :h])
                            nc.sync.dma_start(x_res[:, ko, h:], xT_ap[:, ko, h:])
                        else:
                            nc.sync.dma_start(x_res[:, ko, :], xT_ap[:, ko, :])
                    if interleave_x and ko < 2:
                        h = N_TILE // 2
                        n0 = nt * N_TILE
                        nc.sync.dma_start(
                            w_tile[:, ko, :h], wT_ap[:, ko, n0 : n0 + h]
                        )
                        nc.sync.dma_start(
                            w_tile[:, ko, h:], wT_ap[:, ko, n0 + h : n0 + N_TILE]
                        )
                    else:
                        nc.sync.dma_start(w_tile[:, ko, :], wT_ap[:, ko, nsl])
                    # Binarize in place: bf16 {-1, 0, +1}; exact values.
                    nc.scalar.sign(w_tile[:, ko, :], w_tile[:, ko, :])
                return w_tile

            # First n-tile's weight stream is interleaved with the x
            # residency load so the PE can start as early as possible.
            w0 = load_w(0, interleave_x=True)

            for nt in range(nt_cnt):
                w_tile = w0 if nt == 0 else load_w(nt)
                nsl = slice(nt * N_TILE, (nt + 1) * N_TILE)
                if nt == 0:
                    # k-outer during the ramp: one (x, w) chunk-pair per
                    # k-step feeds 8 matmuls (one per psum bank), so the
                    # PE keeps up with the DMA arrival order.
                    pss = [
                        pspool.tile(
                            [P, N_TILE], mybir.dt.float32, name=f"ps0_{mj}", tag="ps"
                        )
                        for mj in range(mj_cnt)
                    ]
                    for ko in range(ko_cnt):
                        for mj in range(mj_cnt):
                            nc.tensor.matmul(
                                pss[mj][:],
                                x_res[:, ko, mj * P : (mj + 1) * P],
                                w_tile[:, ko, :],
                                start=(ko == 0),
                                stop=(ko == ko_cnt - 1),
                            )
                    for mj in range(mj_cnt):
                        o_t = opool.tile([P, N_TILE], mybir.dt.float32)
                        nc.vector.tensor_copy(out=o_t[:], in_=pss[mj][:])
                        nc.sync.dma_start(out_ap[mj, :, nsl], o_t[:])
                    continue
                for mj in range(mj_cnt):
                    ps = pspool.tile([P, N_TILE], mybir.dt.float32, tag="ps")
                    o_t = opool.tile([P, N_TILE], mybir.dt.float32)
                    if nt == nt_cnt - 1 and mj == mj_cnt - 1:
                        # Kernel-tail drain: run the final tile as two
                        # sequential N=256 accumulation groups, so the
                        # first half's copy + store complete under the
                        # second half's matmuls and only 128KB remains
                        # after the last matmul.
                        h = N_TILE // 2
                        n0 = nt * N_TILE
                        for half in range(2):
                            hs = slice(half * h, (half + 1) * h)
                            for ko in range(ko_cnt):
                                nc.tensor.matmul(
                                    ps[:, hs],
                                    x_res[:, ko, mj * P : (mj + 1) * P],
                                    w_tile[:, ko, hs],
                                    start=(ko == 0),
                                    stop=(ko == ko_cnt - 1),
                                )
                            nc.vector.tensor_copy(out=o_t[:, hs], in_=ps[:, hs])
                            nc.sync.dma_start(
                                out_ap[mj, :, n0 + half * h : n0 + (half + 1) * h],
                                o_t[:, hs],
                            )
                    else:
                        for ko in range(ko_cnt):
                            nc.tensor.matmul(
                                ps[:],
                                x_res[:, ko, mj * P : (mj + 1) * P],
                                w_tile[:, ko, :],
                                start=(ko == 0),
                                stop=(ko == ko_cnt - 1),
                            )
                        nc.vector.tensor_copy(out=o_t[:], in_=ps[:])
                        nc.sync.dma_start(out_ap[mj, :, nsl], o_t[:])

    return nc


_CACHE: dict = {}


def _get_finalized_nc():
    nc = _CACHE.get("nc")
    if nc is None:
        nc = build_nc()
        nc.finalize()
        _CACHE["nc"] = nc
    return nc


def _host_prep(x: np.ndarray, weight: np.ndarray):
    """bf16 cast + K-major transposes. Returns (xT_global [8*K, mc], wT)."""
    mc = M // N_CORES
    # bf16 transposes through uint16 views (vectorized; ml_dtypes object
    # paths can be slow for strided copies).
    x_u16 = np.ascontiguousarray(
        x.astype(BF16).view(np.uint16).reshape(N_CORES, mc, K).transpose(0, 2, 1)
    )
    xt_global = x_u16.reshape(N_CORES * K, mc).view(BF16)
    wt = np.ascontiguousarray(weight.astype(BF16).view(np.uint16).T).view(BF16)
    return xt_global, wt


def make_in_maps(x: np.ndarray, weight: np.ndarray):
    xt_global, wt = _host_prep(x, weight)
    return [
        {"xT": xt_global[c * K : (c + 1) * K], "wT": wt} for c in range(N_CORES)
    ]


def kernel(x: np.ndarray, weight: np.ndarray) -> np.ndarray:
    x = np.asarray(x)
    weight = np.asarray(weight)
    assert x.shape == (M, K) and weight.shape == (N, K)

    nc = _get_finalized_nc()
    from concourse.bass_utils import run_bass_kernel_spmd

    in_maps = make_in_maps(x, weight)
    try:
        res = run_bass_kernel_spmd(nc, in_maps, core_ids=list(range(N_CORES)))
    except Exception:
        # Transient device hiccups (e.g. NRT_EXEC_UNIT_UNRECOVERABLE) have
        # been observed once across many runs; one retry clears them.
        res = run_bass_kernel_spmd(nc, in_maps, core_ids=list(range(N_CORES)))
    out = np.concatenate([res.results[c]["out"] for c in range(N_CORES)], axis=0)
    return np.ascontiguousarray(out.astype(np.float32, copy=False))

